# revision 11
# baseline (speedup 1.0000x reference)
"""Dice loss (sigmoid + per-sample weighted sums) on 8 Trainium2 NeuronCores.

Data-parallel: the flattened per-sample element axis (192^3 = 7,077,888) is
sharded contiguously across 8 cores (884,736 elements = [128 x 6912] each).
Each core computes per-partition partial sums of sigmoid(pred), of
sigmoid(pred)*target, and of target for each of the 3 samples; the host sums
the partials and finishes the dice formula (per the data-parallel hint).

Per-core pipeline (memory-bound; ~21.2 MB HBM traffic/core):
  per chunk: pred DMA on the sync HWDGE ring, target DMA on the scalar HWDGE
  ring (splitting issue across both rings measured faster on HW);
  ScalarE sigmoid with fused per-partition accumulate (sum p);
  VectorE scalar_tensor_tensor p*t with fused accumulate (sum p*t);
  sum t alternates between VectorE tensor_reduce and ScalarE copy+accumulate.
  All partials land in one shared SBUF stats tile -> single output DMA.
  Samples 0-1 use 1728-wide chunks (fewer DMAs); sample 2 uses 864-wide
  chunks so the pipeline tail after the last DMA is shorter.
"""

import numpy as np

import concourse.bacc as bacc
import concourse.tile as tile
from concourse import mybir
from concourse.bass_utils import run_bass_kernel_spmd

B = 3                 # batch (samples)
N_CORES = 8
D = 192
N = D * D * D         # 7,077,888 elements per sample
SHARD = N // N_CORES  # 884,736 per core per sample
P = 128               # SBUF partitions
F = SHARD // P        # 6912 free elements per partition

# chunk plan per sample (each list must sum to F)
PLANS = [[1728] * 4, [1728] * 4, [864] * 8]
NCOLS = sum(len(p) for p in PLANS)          # stat columns per quantity (16)
SAMPLE_COL_OFFSETS = np.cumsum([0] + [len(p) for p in PLANS])  # [0, 4, 8, 16]
MAXC = max(max(p) for p in PLANS)
FP32 = mybir.dt.float32

_nc_cache = None


def _build(repeat=1):
    nc = bacc.Bacc("TRN2")
    pred = nc.dram_tensor("pred", [B, P, F], FP32, kind="ExternalInput")
    targ = nc.dram_tensor("target", [B, P, F], FP32, kind="ExternalInput")
    # out[:, q*NCOLS + k]: q=0 -> sum sigmoid(p), q=1 -> sum p*t, q=2 -> sum t
    out = nc.dram_tensor("out", [P, 3 * NCOLS], FP32, kind="ExternalOutput")

    with tile.TileContext(nc) as tc:
        with (
            tc.tile_pool(name="io", bufs=6) as io,
            tc.tile_pool(name="tmp", bufs=3) as tmp,
            tc.tile_pool(name="stats", bufs=1) as stats,
        ):
            st = stats.tile([P, 3 * NCOLS], FP32, tag="st")
            st_p = st[:, 0:NCOLS]
            st_pt = st[:, NCOLS : 2 * NCOLS]
            st_t = st[:, 2 * NCOLS : 3 * NCOLS]
            for _ in range(repeat):
                k = 0
                for b, plan in enumerate(PLANS):
                    off = 0
                    for ch in plan:
                        p_in = io.tile([P, MAXC], FP32, tag="p_in")
                        t_in = io.tile([P, MAXC], FP32, tag="t_in")
                        cols = slice(off, off + ch)
                        # split input DMA issue across both HWDGE rings
                        nc.sync.dma_start(out=p_in[:, :ch], in_=pred[b, :, cols])
                        nc.scalar.dma_start(out=t_in[:, :ch], in_=targ[b, :, cols])

                        sig = tmp.tile([P, MAXC], FP32, tag="sig")
                        nc.scalar.activation(
                            sig[:, :ch],
                            p_in[:, :ch],
                            mybir.ActivationFunctionType.Sigmoid,
                            accum_out=st_p[:, k : k + 1],
                        )
                        prod = tmp.tile([P, MAXC], FP32, tag="prod")
                        nc.vector.scalar_tensor_tensor(
                            out=prod[:, :ch],
                            in0=sig[:, :ch],
                            scalar=0.0,
                            in1=t_in[:, :ch],
                            op0=mybir.AluOpType.bypass,
                            op1=mybir.AluOpType.mult,
                            accum_out=st_pt[:, k : k + 1],
                        )
                        # balance sum(t) across the two elementwise engines
                        if k % 2 == 0:
                            nc.vector.tensor_reduce(
                                out=st_t[:, k : k + 1],
                                in_=t_in[:, :ch],
                                axis=mybir.AxisListType.X,
                                op=mybir.AluOpType.add,
                            )
                        else:
                            tcopy = tmp.tile([P, MAXC], FP32, tag="tcopy")
                            nc.scalar.activation(
                                tcopy[:, :ch],
                                t_in[:, :ch],
                                mybir.ActivationFunctionType.Copy,
                                accum_out=st_t[:, k : k + 1],
                            )
                        off += ch
                        k += 1
            nc.sync.dma_start(out=out[:, :], in_=st)
    nc.compile()
    return nc


def run(pred, target, weight, **spmd_kwargs):
    global _nc_cache
    if _nc_cache is None:
        _nc_cache = _build()
    nc = _nc_cache

    p2 = np.asarray(pred, dtype=np.float32).reshape(B, N)
    t2 = np.asarray(target, dtype=np.float32).reshape(B, N)
    in_maps = []
    for i in range(N_CORES):
        sl = slice(i * SHARD, (i + 1) * SHARD)
        in_maps.append(
            {
                "pred": np.ascontiguousarray(p2[:, sl]).reshape(B, P, F),
                "target": np.ascontiguousarray(t2[:, sl]).reshape(B, P, F),
            }
        )
    res = run_bass_kernel_spmd(nc, in_maps, core_ids=list(range(N_CORES)), **spmd_kwargs)

    partials = np.stack([r["out"] for r in res.results])  # [8, P, 3*NCOLS]
    grp = partials.reshape(N_CORES, P, 3, NCOLS)
    # per-sample sums over cores, partitions, and that sample's chunk columns
    s_b = np.empty((3, B), dtype=np.float64)
    for b in range(B):
        lo, hi = SAMPLE_COL_OFFSETS[b], SAMPLE_COL_OFFSETS[b + 1]
        s_b[:, b] = grp[:, :, :, lo:hi].sum(axis=(0, 1, 3), dtype=np.float64)
    psum, inter, tsum = s_b[0], s_b[1], s_b[2]
    w = np.asarray(weight, dtype=np.float64)
    smooth = 1.0
    dice = (2.0 * inter * w + smooth) / (psum * w + tsum * w + smooth)
    loss = np.sum(1.0 - dice) / B
    return np.array(loss, dtype=np.float32), res


def kernel(pred, target, weight):
    loss, _ = run(pred, target, weight)
    return loss


# revision 13
# speedup vs baseline: 1.0039x; 1.0039x over previous
"""Dice loss (sigmoid + per-sample weighted sums) on 8 Trainium2 NeuronCores.

Data-parallel: the flattened per-sample element axis (192^3 = 7,077,888) is
sharded contiguously across 8 cores (884,736 elements = [128 x 6912] each).
Each core computes per-partition partial sums of sigmoid(pred), of
sigmoid(pred)*target, and of target for each of the 3 samples; the host sums
the partials and finishes the dice formula (per the data-parallel hint).

Per-core pipeline (memory-bound; ~21.2 MB HBM traffic/core):
  per chunk: pred DMA on the sync HWDGE ring, target DMA on the scalar HWDGE
  ring (splitting issue across both rings measured faster on HW);
  ScalarE sigmoid with fused per-partition accumulate (sum p);
  VectorE scalar_tensor_tensor p*t with fused accumulate (sum p*t);
  sum t alternates between VectorE tensor_reduce and ScalarE copy+accumulate.
  All partials land in one shared SBUF stats tile -> single output DMA.
  Samples 0-1 use 1728-wide chunks (fewer DMAs); sample 2 uses 864-wide
  chunks so the pipeline tail after the last DMA is shorter.
"""

import numpy as np

import concourse.bacc as bacc
import concourse.tile as tile
from concourse import mybir
from concourse.bass_utils import run_bass_kernel_spmd
from concourse.vector_clock import ScopedClock


class _LeanTileContext(tile.TileContext):
    """Tile exit for single-TileContext kernels: drop the trailing all-engine
    barrier (it only fences semaphore reuse by a subsequent TileContext, which
    this kernel doesn't have) and exclude the unused PE engine from the
    pre-clear barrier. NRT re-executes a NEFF only after every engine halted,
    and gpsimd halts after the semaphore clears, so re-execution still sees
    cleared semaphores. Validated on HW over 10 consecutive dispatches of one
    loaded executable."""

    def _drain_and_barrier(self, tick_clock, wait_clock):
        drain_inst = self.nc.sync.drain()
        wait_clock.add_sem_waits(
            drain_inst.ins, ScopedClock({None: tick_clock.global_clock})
        )
        self.nc.multi_engine_barrier(
            [
                mybir.EngineType.SP,
                mybir.EngineType.Activation,
                mybir.EngineType.DVE,
                mybir.EngineType.Pool,
            ]
        )
        popped = self.nc._tile_sem_poison_stack.pop()
        assert popped is self._sem_poison
        self.nc.clear_and_free_semaphores(list(self.sems.allocated().values()))

B = 3                 # batch (samples)
N_CORES = 8
D = 192
N = D * D * D         # 7,077,888 elements per sample
SHARD = N // N_CORES  # 884,736 per core per sample
P = 128               # SBUF partitions
F = SHARD // P        # 6912 free elements per partition

# chunk plan per sample (each list must sum to F)
PLANS = [[1728] * 4, [1728] * 4, [864] * 8]
NCOLS = sum(len(p) for p in PLANS)          # stat columns per quantity (16)
SAMPLE_COL_OFFSETS = np.cumsum([0] + [len(p) for p in PLANS])  # [0, 4, 8, 16]
MAXC = max(max(p) for p in PLANS)
FP32 = mybir.dt.float32

_nc_cache = None


def _build(repeat=1):
    nc = bacc.Bacc("TRN2")
    pred = nc.dram_tensor("pred", [B, P, F], FP32, kind="ExternalInput")
    targ = nc.dram_tensor("target", [B, P, F], FP32, kind="ExternalInput")
    # out[:, q*NCOLS + k]: q=0 -> sum sigmoid(p), q=1 -> sum p*t, q=2 -> sum t
    out = nc.dram_tensor("out", [P, 3 * NCOLS], FP32, kind="ExternalOutput")

    with _LeanTileContext(nc) as tc:
        with (
            tc.tile_pool(name="io", bufs=6) as io,
            tc.tile_pool(name="tmp", bufs=3) as tmp,
            tc.tile_pool(name="stats", bufs=1) as stats,
        ):
            st = stats.tile([P, 3 * NCOLS], FP32, tag="st")
            st_p = st[:, 0:NCOLS]
            st_pt = st[:, NCOLS : 2 * NCOLS]
            st_t = st[:, 2 * NCOLS : 3 * NCOLS]
            for _ in range(repeat):
                k = 0
                for b, plan in enumerate(PLANS):
                    off = 0
                    for ch in plan:
                        p_in = io.tile([P, MAXC], FP32, tag="p_in")
                        t_in = io.tile([P, MAXC], FP32, tag="t_in")
                        cols = slice(off, off + ch)
                        # split input DMA issue across both HWDGE rings
                        nc.sync.dma_start(out=p_in[:, :ch], in_=pred[b, :, cols])
                        nc.scalar.dma_start(out=t_in[:, :ch], in_=targ[b, :, cols])

                        sig = tmp.tile([P, MAXC], FP32, tag="sig")
                        nc.scalar.activation(
                            sig[:, :ch],
                            p_in[:, :ch],
                            mybir.ActivationFunctionType.Sigmoid,
                            accum_out=st_p[:, k : k + 1],
                        )
                        prod = tmp.tile([P, MAXC], FP32, tag="prod")
                        nc.vector.scalar_tensor_tensor(
                            out=prod[:, :ch],
                            in0=sig[:, :ch],
                            scalar=0.0,
                            in1=t_in[:, :ch],
                            op0=mybir.AluOpType.bypass,
                            op1=mybir.AluOpType.mult,
                            accum_out=st_pt[:, k : k + 1],
                        )
                        # balance sum(t) across the two elementwise engines
                        if k % 2 == 0:
                            nc.vector.tensor_reduce(
                                out=st_t[:, k : k + 1],
                                in_=t_in[:, :ch],
                                axis=mybir.AxisListType.X,
                                op=mybir.AluOpType.add,
                            )
                        else:
                            tcopy = tmp.tile([P, MAXC], FP32, tag="tcopy")
                            nc.scalar.activation(
                                tcopy[:, :ch],
                                t_in[:, :ch],
                                mybir.ActivationFunctionType.Copy,
                                accum_out=st_t[:, k : k + 1],
                            )
                        off += ch
                        k += 1
            nc.sync.dma_start(out=out[:, :], in_=st)
    nc.compile()
    return nc


def run(pred, target, weight, **spmd_kwargs):
    global _nc_cache
    if _nc_cache is None:
        _nc_cache = _build()
    nc = _nc_cache

    p2 = np.asarray(pred, dtype=np.float32).reshape(B, N)
    t2 = np.asarray(target, dtype=np.float32).reshape(B, N)
    in_maps = []
    for i in range(N_CORES):
        sl = slice(i * SHARD, (i + 1) * SHARD)
        in_maps.append(
            {
                "pred": np.ascontiguousarray(p2[:, sl]).reshape(B, P, F),
                "target": np.ascontiguousarray(t2[:, sl]).reshape(B, P, F),
            }
        )
    res = run_bass_kernel_spmd(nc, in_maps, core_ids=list(range(N_CORES)), **spmd_kwargs)

    partials = np.stack([r["out"] for r in res.results])  # [8, P, 3*NCOLS]
    grp = partials.reshape(N_CORES, P, 3, NCOLS)
    # per-sample sums over cores, partitions, and that sample's chunk columns
    s_b = np.empty((3, B), dtype=np.float64)
    for b in range(B):
        lo, hi = SAMPLE_COL_OFFSETS[b], SAMPLE_COL_OFFSETS[b + 1]
        s_b[:, b] = grp[:, :, :, lo:hi].sum(axis=(0, 1, 3), dtype=np.float64)
    psum, inter, tsum = s_b[0], s_b[1], s_b[2]
    w = np.asarray(weight, dtype=np.float64)
    smooth = 1.0
    dice = (2.0 * inter * w + smooth) / (psum * w + tsum * w + smooth)
    loss = np.sum(1.0 - dice) / B
    return np.array(loss, dtype=np.float32), res


def kernel(pred, target, weight):
    loss, _ = run(pred, target, weight)
    return loss


# revision 15
# speedup vs baseline: 1.0081x; 1.0041x over previous
"""Dice loss (sigmoid + per-sample weighted sums) on 8 Trainium2 NeuronCores.

Data-parallel: the flattened per-sample element axis (192^3 = 7,077,888) is
sharded contiguously across 8 cores (884,736 elements = [128 x 6912] each).
Each core computes per-partition partial sums of sigmoid(pred), of
sigmoid(pred)*target, and of target for each of the 3 samples; the host sums
the partials and finishes the dice formula (per the data-parallel hint).

Per-core pipeline (memory-bound; ~21.2 MB HBM traffic/core):
  per chunk: pred DMA on the sync HWDGE ring, target DMA on the scalar HWDGE
  ring (splitting issue across both rings measured faster on HW);
  ScalarE sigmoid with fused per-partition accumulate (sum p);
  VectorE scalar_tensor_tensor p*t with fused accumulate (sum p*t);
  sum t alternates between VectorE tensor_reduce and ScalarE copy+accumulate.
  All partials land in one shared SBUF stats tile -> single output DMA.
  Samples 0-1 use 1728-wide chunks (fewer DMAs); sample 2 uses 864-wide
  chunks so the pipeline tail after the last DMA is shorter.
"""

import numpy as np

import concourse.bacc as bacc
import concourse.tile as tile
from concourse import mybir
from concourse.bass_utils import run_bass_kernel_spmd
from concourse.vector_clock import ScopedClock


class _LeanTileContext(tile.TileContext):
    """Tile exit for single-TileContext kernels, three changes vs stock:

    1. The final output DMA is issued here, between the drain and the barrier,
       on a non-Tile semaphore — its ~1.5 us HBM write receipt then overlaps
       the exit barrier and the semaphore clears instead of serializing before
       them. gpsimd waits the receipt last and resets the semaphore so
       re-execution of the loaded NEFF sees a clean state.
    2. The trailing all-engine barrier is dropped (it only fences semaphore
       reuse by a subsequent TileContext, which this kernel doesn't have).
    3. The unused PE engine is excluded from the pre-clear barrier.

    NRT re-executes a NEFF only after every engine halted, and gpsimd halts
    after the clears + receipt wait, so re-execution is safe. Validated on HW
    over 10 consecutive dispatches of one loaded executable."""

    final_dma = None  # (out_dram_ap, stats_tile_ap) set by _build

    def _drain_and_barrier(self, tick_clock, wait_clock):
        nc = self.nc
        drain_inst = nc.sync.drain()
        wait_clock.add_sem_waits(
            drain_inst.ins, ScopedClock({None: tick_clock.global_clock})
        )
        out_sem = None
        if self.final_dma is not None:
            out_ap, in_ap = self.final_dma
            if self.is_my_tile(in_ap.tensor):
                in_ap.tensor = in_ap.tensor.concrete_tensor()
            out_sem = nc.alloc_semaphore("final_out_dma_sem")
            nc.sync.dma_start(out=out_ap, in_=in_ap).then_inc(out_sem, 16)
        nc.multi_engine_barrier(
            [
                mybir.EngineType.SP,
                mybir.EngineType.Activation,
                mybir.EngineType.DVE,
                mybir.EngineType.Pool,
            ]
        )
        popped = nc._tile_sem_poison_stack.pop()
        assert popped is self._sem_poison
        nc.clear_and_free_semaphores(list(self.sems.allocated().values()))
        if out_sem is not None:
            nc.gpsimd.wait_ge(out_sem, 16)
            nc.gpsimd.sem_clear(out_sem)

B = 3                 # batch (samples)
N_CORES = 8
D = 192
N = D * D * D         # 7,077,888 elements per sample
SHARD = N // N_CORES  # 884,736 per core per sample
P = 128               # SBUF partitions
F = SHARD // P        # 6912 free elements per partition

# chunk plan per sample (each list must sum to F)
PLANS = [[1728] * 4, [1728] * 4, [864] * 8]
NCOLS = sum(len(p) for p in PLANS)          # stat columns per quantity (16)
SAMPLE_COL_OFFSETS = np.cumsum([0] + [len(p) for p in PLANS])  # [0, 4, 8, 16]
MAXC = max(max(p) for p in PLANS)
FP32 = mybir.dt.float32

_nc_cache = None


def _build(repeat=1):
    nc = bacc.Bacc("TRN2")
    pred = nc.dram_tensor("pred", [B, P, F], FP32, kind="ExternalInput")
    targ = nc.dram_tensor("target", [B, P, F], FP32, kind="ExternalInput")
    # out[:, q*NCOLS + k]: q=0 -> sum sigmoid(p), q=1 -> sum p*t, q=2 -> sum t
    out = nc.dram_tensor("out", [P, 3 * NCOLS], FP32, kind="ExternalOutput")

    with _LeanTileContext(nc) as tc:
        with (
            tc.tile_pool(name="io", bufs=6) as io,
            tc.tile_pool(name="tmp", bufs=3) as tmp,
            tc.tile_pool(name="stats", bufs=1) as stats,
        ):
            st = stats.tile([P, 3 * NCOLS], FP32, tag="st")
            st_p = st[:, 0:NCOLS]
            st_pt = st[:, NCOLS : 2 * NCOLS]
            st_t = st[:, 2 * NCOLS : 3 * NCOLS]
            for _ in range(repeat):
                k = 0
                for b, plan in enumerate(PLANS):
                    off = 0
                    for ch in plan:
                        p_in = io.tile([P, MAXC], FP32, tag="p_in")
                        t_in = io.tile([P, MAXC], FP32, tag="t_in")
                        cols = slice(off, off + ch)
                        # split input DMA issue across both HWDGE rings
                        nc.sync.dma_start(out=p_in[:, :ch], in_=pred[b, :, cols])
                        nc.scalar.dma_start(out=t_in[:, :ch], in_=targ[b, :, cols])

                        sig = tmp.tile([P, MAXC], FP32, tag="sig")
                        nc.scalar.activation(
                            sig[:, :ch],
                            p_in[:, :ch],
                            mybir.ActivationFunctionType.Sigmoid,
                            accum_out=st_p[:, k : k + 1],
                        )
                        prod = tmp.tile([P, MAXC], FP32, tag="prod")
                        nc.vector.scalar_tensor_tensor(
                            out=prod[:, :ch],
                            in0=sig[:, :ch],
                            scalar=0.0,
                            in1=t_in[:, :ch],
                            op0=mybir.AluOpType.bypass,
                            op1=mybir.AluOpType.mult,
                            accum_out=st_pt[:, k : k + 1],
                        )
                        # balance sum(t) across the two elementwise engines
                        if k % 2 == 0:
                            nc.vector.tensor_reduce(
                                out=st_t[:, k : k + 1],
                                in_=t_in[:, :ch],
                                axis=mybir.AxisListType.X,
                                op=mybir.AluOpType.add,
                            )
                        else:
                            tcopy = tmp.tile([P, MAXC], FP32, tag="tcopy")
                            nc.scalar.activation(
                                tcopy[:, :ch],
                                t_in[:, :ch],
                                mybir.ActivationFunctionType.Copy,
                                accum_out=st_t[:, k : k + 1],
                            )
                        off += ch
                        k += 1
            # emitted by _LeanTileContext._drain_and_barrier so the DMA's HBM
            # write receipt overlaps the exit barrier and semaphore clears
            tc.final_dma = (out[:, :], st[:, :])
    nc.compile()
    return nc


def run(pred, target, weight, **spmd_kwargs):
    global _nc_cache
    if _nc_cache is None:
        _nc_cache = _build()
    nc = _nc_cache

    p2 = np.asarray(pred, dtype=np.float32).reshape(B, N)
    t2 = np.asarray(target, dtype=np.float32).reshape(B, N)
    in_maps = []
    for i in range(N_CORES):
        sl = slice(i * SHARD, (i + 1) * SHARD)
        in_maps.append(
            {
                "pred": np.ascontiguousarray(p2[:, sl]).reshape(B, P, F),
                "target": np.ascontiguousarray(t2[:, sl]).reshape(B, P, F),
            }
        )
    res = run_bass_kernel_spmd(nc, in_maps, core_ids=list(range(N_CORES)), **spmd_kwargs)

    partials = np.stack([r["out"] for r in res.results])  # [8, P, 3*NCOLS]
    grp = partials.reshape(N_CORES, P, 3, NCOLS)
    # per-sample sums over cores, partitions, and that sample's chunk columns
    s_b = np.empty((3, B), dtype=np.float64)
    for b in range(B):
        lo, hi = SAMPLE_COL_OFFSETS[b], SAMPLE_COL_OFFSETS[b + 1]
        s_b[:, b] = grp[:, :, :, lo:hi].sum(axis=(0, 1, 3), dtype=np.float64)
    psum, inter, tsum = s_b[0], s_b[1], s_b[2]
    w = np.asarray(weight, dtype=np.float64)
    smooth = 1.0
    dice = (2.0 * inter * w + smooth) / (psum * w + tsum * w + smooth)
    loss = np.sum(1.0 - dice) / B
    return np.array(loss, dtype=np.float32), res


def kernel(pred, target, weight):
    loss, _ = run(pred, target, weight)
    return loss


# revision 17
# speedup vs baseline: 1.0229x; 1.0147x over previous
"""Dice loss (sigmoid + per-sample weighted sums) on 8 Trainium2 NeuronCores.

Data-parallel: the flattened per-sample element axis (192^3 = 7,077,888) is
sharded contiguously across 8 cores (884,736 elements = [128 x 6912] each).
Each core computes per-partition partial sums of sigmoid(pred), of
sigmoid(pred)*target, and of target for each of the 3 samples; the host sums
the partials and finishes the dice formula (per the data-parallel hint).

Per-core pipeline (memory-bound; ~21.2 MB HBM traffic/core):
  per chunk: pred DMA on the sync HWDGE ring, target DMA on the scalar HWDGE
  ring (splitting issue across both rings measured faster on HW);
  ScalarE sigmoid with fused per-partition accumulate (sum p);
  VectorE scalar_tensor_tensor p*t with fused accumulate (sum p*t);
  sum t alternates between VectorE tensor_reduce and ScalarE copy+accumulate.
  All partials land in one shared SBUF stats tile -> single output DMA.
  Samples 0-1 use 1728-wide chunks (fewer DMAs); sample 2 uses 864-wide
  chunks so the pipeline tail after the last DMA is shorter.
"""

import numpy as np

import concourse.bacc as bacc
import concourse.tile as tile
from concourse import mybir
from concourse.bass_utils import run_bass_kernel_spmd
from concourse.vector_clock import ScopedClock


class _LeanTileContext(tile.TileContext):
    """Tile exit for single-TileContext kernels, three changes vs stock:

    1. The final output DMA is issued here, between the drain and the barrier,
       on a non-Tile semaphore — its ~1.5 us HBM write receipt then overlaps
       the exit barrier and the semaphore clears instead of serializing before
       them. gpsimd waits the receipt last and resets the semaphore so
       re-execution of the loaded NEFF sees a clean state.
    2. The trailing all-engine barrier is dropped (it only fences semaphore
       reuse by a subsequent TileContext, which this kernel doesn't have).
    3. The unused PE engine is excluded from the pre-clear barrier.

    NRT re-executes a NEFF only after every engine halted, and gpsimd halts
    after the clears + receipt wait, so re-execution is safe. Validated on HW
    over 10 consecutive dispatches of one loaded executable."""

    final_dma = None  # (out_dram_ap, stats_tile_ap) set by _build

    def _drain_and_barrier(self, tick_clock, wait_clock):
        nc = self.nc
        drain_inst = nc.sync.drain()
        wait_clock.add_sem_waits(
            drain_inst.ins, ScopedClock({None: tick_clock.global_clock})
        )
        out_sem = None
        if self.final_dma is not None:
            out_ap, in_ap = self.final_dma
            if self.is_my_tile(in_ap.tensor):
                in_ap.tensor = in_ap.tensor.concrete_tensor()
            out_sem = nc.alloc_semaphore("final_out_dma_sem")
            nc.sync.dma_start(out=out_ap, in_=in_ap).then_inc(out_sem, 16)
        nc.multi_engine_barrier(
            [
                mybir.EngineType.SP,
                mybir.EngineType.Activation,
                mybir.EngineType.DVE,
                mybir.EngineType.Pool,
            ]
        )
        popped = nc._tile_sem_poison_stack.pop()
        assert popped is self._sem_poison
        nc.clear_and_free_semaphores(list(self.sems.allocated().values()))
        if out_sem is not None:
            nc.gpsimd.wait_ge(out_sem, 16)
            nc.gpsimd.sem_clear(out_sem)

B = 3                 # batch (samples)
N_CORES = 8
D = 192
N = D * D * D         # 7,077,888 elements per sample
SHARD = N // N_CORES  # 884,736 per core per sample
P = 128               # SBUF partitions
F = SHARD // P        # 6912 free elements per partition

# chunk plan per sample (each list must sum to F)
PLANS = [[1728] * 4, [1728] * 4, [864] * 8]
NCOLS = sum(len(p) for p in PLANS)          # stat columns per quantity (16)
SAMPLE_COL_OFFSETS = np.cumsum([0] + [len(p) for p in PLANS])  # [0, 4, 8, 16]
MAXC = max(max(p) for p in PLANS)
FP32 = mybir.dt.float32

_nc_cache = None


def _build(repeat=1):
    nc = bacc.Bacc("TRN2")
    pred = nc.dram_tensor("pred", [B, P, F], FP32, kind="ExternalInput")
    targ = nc.dram_tensor("target", [B, P, F], FP32, kind="ExternalInput")
    # out[:, q*NCOLS + k]: q=0 -> sum sigmoid(p), q=1 -> sum p*t, q=2 -> sum t
    out = nc.dram_tensor("out", [P, 3 * NCOLS], FP32, kind="ExternalOutput")

    with _LeanTileContext(nc) as tc:
        with (
            tc.tile_pool(name="io", bufs=6) as io,
            tc.tile_pool(name="tmp", bufs=3) as tmp,
            tc.tile_pool(name="ps", bufs=2, space="PSUM") as ps,
            tc.tile_pool(name="const", bufs=1) as const,
            tc.tile_pool(name="stats", bufs=1) as stats,
        ):
            ones = const.tile([P, 1], FP32, tag="ones")
            nc.vector.memset(ones, 1.0)
            st = stats.tile([P, 3 * NCOLS], FP32, tag="st")
            st_p = st[:, 0:NCOLS]
            st_pt = st[:, NCOLS : 2 * NCOLS]
            st_t = st[:, 2 * NCOLS : 3 * NCOLS]
            for _ in range(repeat):
                k = 0
                for b, plan in enumerate(PLANS):
                    off = 0
                    for ch in plan:
                        p_in = io.tile([P, MAXC], FP32, tag="p_in")
                        t_in = io.tile([P, MAXC], FP32, tag="t_in")
                        cols = slice(off, off + ch)
                        # split input DMA issue across both HWDGE rings
                        nc.sync.dma_start(out=p_in[:, :ch], in_=pred[b, :, cols])
                        nc.scalar.dma_start(out=t_in[:, :ch], in_=targ[b, :, cols])

                        sig = tmp.tile([P, MAXC], FP32, tag="sig")
                        nc.scalar.activation(
                            sig[:, :ch],
                            p_in[:, :ch],
                            mybir.ActivationFunctionType.Sigmoid,
                            accum_out=st_p[:, k : k + 1],
                        )
                        prod = tmp.tile([P, MAXC], FP32, tag="prod")
                        nc.vector.scalar_tensor_tensor(
                            out=prod[:, :ch],
                            in0=sig[:, :ch],
                            scalar=0.0,
                            in1=t_in[:, :ch],
                            op0=mybir.AluOpType.bypass,
                            op1=mybir.AluOpType.mult,
                            accum_out=st_pt[:, k : k + 1],
                        )
                        # sum(t) on the otherwise-idle TensorEngine: 128-column
                        # blocks of t times a ones vector, accumulated into one
                        # PSUM column. The 128 entries are per-PE-row partials;
                        # the host sums them with all other partials, so no
                        # further on-chip reduction is needed.
                        acc = ps.tile([P, 1], FP32, tag="acc")
                        m_offs = list(range(0, ch, P))
                        for i, m0 in enumerate(m_offs):
                            m = min(P, ch - m0)
                            nc.tensor.matmul(
                                acc[:m, 0:1],
                                t_in[:, m0 : m0 + m],
                                ones[:, 0:1],
                                start=(i == 0),
                                stop=(i == len(m_offs) - 1),
                            )
                        nc.vector.tensor_copy(st_t[:, k : k + 1], acc[:, 0:1])
                        off += ch
                        k += 1
            # emitted by _LeanTileContext._drain_and_barrier so the DMA's HBM
            # write receipt overlaps the exit barrier and semaphore clears
            tc.final_dma = (out[:, :], st[:, :])
    nc.compile()
    return nc


def run(pred, target, weight, **spmd_kwargs):
    global _nc_cache
    if _nc_cache is None:
        _nc_cache = _build()
    nc = _nc_cache

    p2 = np.asarray(pred, dtype=np.float32).reshape(B, N)
    t2 = np.asarray(target, dtype=np.float32).reshape(B, N)
    in_maps = []
    for i in range(N_CORES):
        sl = slice(i * SHARD, (i + 1) * SHARD)
        in_maps.append(
            {
                "pred": np.ascontiguousarray(p2[:, sl]).reshape(B, P, F),
                "target": np.ascontiguousarray(t2[:, sl]).reshape(B, P, F),
            }
        )
    res = run_bass_kernel_spmd(nc, in_maps, core_ids=list(range(N_CORES)), **spmd_kwargs)

    partials = np.stack([r["out"] for r in res.results])  # [8, P, 3*NCOLS]
    grp = partials.reshape(N_CORES, P, 3, NCOLS)
    # per-sample sums over cores, partitions, and that sample's chunk columns
    s_b = np.empty((3, B), dtype=np.float64)
    for b in range(B):
        lo, hi = SAMPLE_COL_OFFSETS[b], SAMPLE_COL_OFFSETS[b + 1]
        s_b[:, b] = grp[:, :, :, lo:hi].sum(axis=(0, 1, 3), dtype=np.float64)
    psum, inter, tsum = s_b[0], s_b[1], s_b[2]
    w = np.asarray(weight, dtype=np.float64)
    smooth = 1.0
    dice = (2.0 * inter * w + smooth) / (psum * w + tsum * w + smooth)
    loss = np.sum(1.0 - dice) / B
    return np.array(loss, dtype=np.float32), res


def kernel(pred, target, weight):
    loss, _ = run(pred, target, weight)
    return loss
